# revision 23
# baseline (speedup 1.0000x reference)
"""Pointer-generator decoder step on 8 Trainium2 NeuronCores.

Strategy:
  - Data-parallel over batch (8 batches/core) for attention + LSTM + scatter.
  - Tensor-parallel over vocab for the v_w projection (each core computes a
    6656-wide logits chunk for all 64 batches), AllGather(h_t) before it and
    AllToAll(logits) after it so each core mixes + writes full-vocab rows for
    its own 8 batches.
  - The scatter-add of attention weights into the vocab dim is computed as
    attn_scores[hi, lo] = sum_s a[s] * [idx[s]>>9 == hi] * [idx[s]&511 == lo]
    i.e. one PE matmul per 128-index tile with fp16 one-hot factors built on
    DVE/GPSIMD; duplicates accumulate exactly in fp32 PSUM.

Host-side work is limited to layout glue: transposes/padding/dtype casts of
inputs, sharding, and concatenation of per-core outputs.
"""

import numpy as np
import ml_dtypes

B, S, E, H, EMB, V = 64, 2048, 1024, 512, 256, 50257
NCORES = 8
BL = B // NCORES          # local batches per core
LO = 512                  # low one-hot width (vocab = hi*512 + lo)
HI = 104                  # hi rows in the [HI, LO] vocab grid (99 used)
VP = HI * LO              # 53248 padded vocab
VC = VP // NCORES         # 6656 vocab chunk per core (13 x 512)
NVC = VC // 512           # 13 chunks of 512
A2A_W = VC + 16           # a2a row width; col VC holds the partial denominator
ET = E // 128             # 8 e-tiles
HT = H // 128             # 4 h-tiles
ST = S // 128             # 16 s-tiles
SC = S // 512             # 4 s-chunks
NEG = -1e30

_CACHE = {}


def _build_module(phases=4, nb=BL, alimit=99):
    import concourse.bacc as bacc
    import concourse.bass as bass
    import concourse.mybir as mybir
    import concourse.tile as tile
    from concourse.bass import IndirectOffsetOnAxis
    from concourse.masks import make_identity

    dt = mybir.dt
    F32, F16, BF16, I32 = dt.float32, dt.float16, dt.bfloat16, dt.int32
    AF = mybir.ActivationFunctionType
    ALU = mybir.AluOpType
    AX = mybir.AxisListType

    nc = bacc.Bacc("TRN2", target_bir_lowering=False, debug=False,
                   num_devices=NCORES)

    # ---- kernel I/O (per core) ----
    encT = nc.dram_tensor("encT", [BL, E, S], F32, kind="ExternalInput")
    h0T_d = nc.dram_tensor("h0T", [H, BL], F32, kind="ExternalInput")
    c0T_d = nc.dram_tensor("c0T", [H, BL], F32, kind="ExternalInput")
    dec_d = nc.dram_tensor("decin", [BL, 1], I32, kind="ExternalInput")
    eidx_d = nc.dram_tensor("encidx", [BL, S], I32, kind="ExternalInput")
    emb_d = nc.dram_tensor("embed", [V, EMB], F32, kind="ExternalInput")
    whwT_d = nc.dram_tensor("whwT", [E, H], F32, kind="ExternalInput")
    wswT_d = nc.dram_tensor("wswT", [H, H], F32, kind="ExternalInput")
    attnb_d = nc.dram_tensor("attnb", [H, 1], F32, kind="ExternalInput")
    attnv_d = nc.dram_tensor("attnv", [H, 1], F32, kind="ExternalInput")
    wihT_d = nc.dram_tensor("wihT", [E + EMB, 4 * H], F32, kind="ExternalInput")
    whhT_d = nc.dram_tensor("whhT", [H, 4 * H], F32, kind="ExternalInput")
    lstmb_d = nc.dram_tensor("lstmb", [4 * H, 1], F32, kind="ExternalInput")
    whv_d = nc.dram_tensor("whv", [E, 1], F32, kind="ExternalInput")
    wsv_d = nc.dram_tensor("wsv", [H, 1], F32, kind="ExternalInput")
    wxv_d = nc.dram_tensor("wxv", [EMB, 1], F32, kind="ExternalInput")
    vwT_d = nc.dram_tensor("vwT", [H, VC], BF16, kind="ExternalInput")
    vb_d = nc.dram_tensor("vb", [1, VC], BF16, kind="ExternalInput")
    out_d = nc.dram_tensor("out", [BL, VP], F32, kind="ExternalOutput")
    outht_d = nc.dram_tensor("out_ht", [BL, H], F32, kind="ExternalOutput")
    outct_d = nc.dram_tensor("out_ct", [BL, H], F32, kind="ExternalOutput")

    # internal DRAM for collectives + small bounces
    ht_in = nc.dram_tensor("ht_in", [BL, H], F32)
    ht_out = nc.dram_tensor("ht_out", [B, H], F32, addr_space="Shared")
    a2a_in = nc.dram_tensor("a2a_in", [B, A2A_W], F32)
    a2a_out = nc.dram_tensor("a2a_out", [B, A2A_W], F32)
    arow_d = nc.dram_tensor("arow_bounce", [BL, S], F32)

    groups = [list(range(NCORES))]

    with tile.TileContext(nc) as tc:
        with (
            tc.tile_pool(name="const", bufs=1) as constp,
            tc.tile_pool(name="enc", bufs=8) as encp,
            tc.tile_pool(name="tanh", bufs=4) as tanhp,
            tc.tile_pool(name="rows", bufs=2) as rowp,
            tc.tile_pool(name="abc", bufs=2) as abcp,
            tc.tile_pool(name="scr", bufs=2) as scrp,
            tc.tile_pool(name="idx", bufs=2) as idxp,
            tc.tile_pool(name="f16", bufs=4) as f16p,
            tc.tile_pool(name="keep", bufs=1) as keepp,
            tc.tile_pool(name="wtile", bufs=6) as wtp,
            tc.tile_pool(name="mix", bufs=2) as mixp,
            tc.tile_pool(name="ps", bufs=2, space="PSUM") as psp,
            tc.tile_pool(name="ps2", bufs=2, space="PSUM") as ps2p,
        ):
            # ======== constants ========
            ident = constp.tile([128, 128], F32)
            make_identity(nc, ident[:])
            iota_hi32 = constp.tile([128, HI], I32)
            nc.gpsimd.iota(iota_hi32[:], pattern=[[1, HI]], base=0,
                           channel_multiplier=0)
            iota_hi = constp.tile([128, HI], F16)
            nc.vector.tensor_copy(iota_hi[:], iota_hi32[:])
            iota_lo32 = constp.tile([128, LO], I32)
            nc.gpsimd.iota(iota_lo32[:], pattern=[[1, LO]], base=0,
                           channel_multiplier=0)
            iota_lo = constp.tile([128, LO], F16)
            nc.vector.tensor_copy(iota_lo[:], iota_lo32[:])
            ones_bf = constp.tile([1, 64], BF16)
            nc.vector.memset(ones_bf[:], 1.0)

            whw_sb = constp.tile([128, ET * H], F32)       # e-tile j at cols j*H
            for j in range(ET):
                nc.sync.dma_start(whw_sb[:, j * H:(j + 1) * H],
                                  whwT_d[128 * j:128 * (j + 1), :])
            attnb_sb = constp.tile([128, HT], F32)
            attnv_sb = constp.tile([128, HT], F32)
            for j in range(HT):
                nc.sync.dma_start(attnb_sb[:, j:j + 1],
                                  attnb_d[128 * j:128 * (j + 1), :])
                nc.sync.dma_start(attnv_sb[:, j:j + 1],
                                  attnv_d[128 * j:128 * (j + 1), :])
            h0T_sb = constp.tile([128, HT * BL], F32)
            c0T_sb = constp.tile([128, HT * BL], F32)
            for j in range(HT):
                nc.sync.dma_start(h0T_sb[:, BL * j:BL * (j + 1)],
                                  h0T_d[128 * j:128 * (j + 1), :])
                nc.sync.dma_start(c0T_sb[:, BL * j:BL * (j + 1)],
                                  c0T_d[128 * j:128 * (j + 1), :])
            lstmb_sb = constp.tile([128, 16], F32)
            for g in range(16):
                nc.sync.dma_start(lstmb_sb[:, g:g + 1],
                                  lstmb_d[128 * g:128 * (g + 1), :])
            whv_sb = constp.tile([128, ET], F32)
            for j in range(ET):
                nc.sync.dma_start(whv_sb[:, j:j + 1],
                                  whv_d[128 * j:128 * (j + 1), :])
            wsv_sb = constp.tile([128, HT], F32)
            for j in range(HT):
                nc.sync.dma_start(wsv_sb[:, j:j + 1],
                                  wsv_d[128 * j:128 * (j + 1), :])
            wxv_sb = constp.tile([128, 2], F32)
            for j in range(2):
                nc.sync.dma_start(wxv_sb[:, j:j + 1],
                                  wxv_d[128 * j:128 * (j + 1), :])
            # embedding rows for dec_input + transpose to [EMB, BL] columns
            dec_sb = constp.tile([BL, 1], I32)
            nc.sync.dma_start(dec_sb[:], dec_d[:, :])
            emb_sb = constp.tile([BL, EMB], F32)
            nc.gpsimd.indirect_dma_start(
                out=emb_sb[:], out_offset=None, in_=emb_d[:, :],
                in_offset=IndirectOffsetOnAxis(ap=dec_sb[:, :1], axis=0))
            # xT columns: [ctx e-tiles 0..7 | emb tiles 8..9], each [128, BL]
            xT_sb = constp.tile([128, (ET + 2) * BL], F32)
            for j in range(2):
                pst = ps2p.tile([128, BL], F32, tag="small")
                nc.tensor.transpose(pst[:], emb_sb[:, 128 * j:128 * (j + 1)],
                                    ident[:BL, :BL])
                nc.scalar.copy(xT_sb[:, (ET + j) * BL:(ET + j + 1) * BL], pst[:])

            # ws_app (+ combined attention bias) as per-partition columns
            tb_sb = constp.tile([128, HT * BL], F32)
            for ho in range(HT):
                psw = ps2p.tile([128, BL], F32, tag="small")
                for f in range(HT):
                    wsw_t = wtp.tile([128, 128], F32, tag="wsw")
                    nc.sync.dma_start(
                        wsw_t[:], wswT_d[128 * f:128 * (f + 1),
                                         128 * ho:128 * (ho + 1)])
                    nc.tensor.matmul(
                        psw[:], lhsT=wsw_t[:],
                        rhs=h0T_sb[:, BL * f:BL * (f + 1)],
                        start=(f == 0), stop=(f == HT - 1))
                nc.scalar.activation(tb_sb[:, BL * ho:BL * (ho + 1)], psw[:],
                                     AF.Identity, bias=attnb_sb[:, ho:ho + 1])

            # per-b keep tiles
            scat_sb = keepp.tile([128, BL * LO], F32)   # scatter result per b
            lnpg_row = constp.tile([1, BL], F32)
            s2_row = constp.tile([1, BL], F32)

            # ======== phase A: per-batch attention ========
            for b in range(nb):
                enc_tiles = []
                for e in range(ET):
                    t = encp.tile([128, S], F32, tag="enc")
                    nc.sync.dma_start(t[:], encT[b, 128 * e:128 * (e + 1), :])
                    enc_tiles.append(t)

                energy_row = rowp.tile([1, S], F32, tag="energy")
                for sc in range(SC):
                    ps_d = psp.tile([1, 512], F32, tag="psd")
                    for h in range(HT):
                        ps_e = psp.tile([128, 512], F32, tag="pse")
                        for e in range(ET):
                            nc.tensor.matmul(
                                ps_e[:],
                                lhsT=whw_sb[:, H * e + 128 * h:H * e + 128 * (h + 1)],
                                rhs=enc_tiles[e][:, 512 * sc:512 * (sc + 1)],
                                start=(e == 0), stop=(e == ET - 1))
                        th = tanhp.tile([128, 512], F32, tag="tanh")
                        nc.scalar.activation(th[:], ps_e[:], AF.Tanh,
                                             bias=tb_sb[:, BL * h + b:BL * h + b + 1])
                        nc.tensor.matmul(ps_d[:], lhsT=attnv_sb[:, h:h + 1],
                                         rhs=th[:], start=(h == 0),
                                         stop=(h == HT - 1))
                    nc.scalar.copy(energy_row[:, 512 * sc:512 * (sc + 1)], ps_d[:])

                if alimit < 2:
                    continue
                # softmax over s for this batch row
                nmax = rowp.tile([1, 1], F32, tag="nmax")
                nc.vector.tensor_reduce(nmax[:], energy_row[:], axis=AX.X,
                                        op=ALU.max, negate=True)
                esum = rowp.tile([1, 1], F32, tag="esum")
                attn_row = energy_row  # in-place exp
                nc.scalar.activation(attn_row[:], energy_row[:], AF.Exp,
                                     bias=nmax[:, :1], accum_out=esum[:])
                rsum = rowp.tile([1, 1], F32, tag="rsum")
                nc.vector.reciprocal(rsum[:], esum[:])
                nc.vector.tensor_scalar_mul(attn_row[:], attn_row[:], rsum[:, :1])

                if alimit < 3:
                    continue
                # broadcast attn across partitions for the context reduction
                a_bc = abcp.tile([128, S], F32, tag="abc")
                nc.gpsimd.partition_broadcast(a_bc[:], attn_row[:])
                for e in range(ET):
                    # in-place: overwrites the enc tile, which has no later use
                    nc.vector.tensor_tensor(
                        out=enc_tiles[e][:], in0=enc_tiles[e][:], in1=a_bc[:],
                        op=ALU.mult)
                    nc.scalar.activation(
                        enc_tiles[e][:], enc_tiles[e][:], AF.Copy,
                        accum_out=xT_sb[:, BL * e + b:BL * e + b + 1])

                if alimit < 4:
                    continue
                # attn values as [128, ST] columns via DRAM bounce
                nc.sync.dma_start(arow_d[b:b + 1, :], attn_row[:1, :])
                a_cols = idxp.tile([128, ST], F32, tag="acols")
                nc.sync.dma_start(
                    a_cols[:], arow_d[b, :].rearrange("(p t) -> p t", t=ST))

                if alimit < 5:
                    continue
                idx_cols = idxp.tile([128, ST], I32, tag="idxc")
                nc.sync.dma_start(
                    idx_cols[:], eidx_d[b, :].rearrange("(p t) -> p t", t=ST))
                hi32 = idxp.tile([128, ST], I32, tag="hi32")
                nc.vector.tensor_scalar(hi32[:], idx_cols[:], 9, None,
                                        ALU.logical_shift_right)
                lo32 = idxp.tile([128, ST], I32, tag="lo32")
                nc.vector.tensor_scalar(lo32[:], idx_cols[:], 511, None,
                                        ALU.bitwise_and)
                hif = idxp.tile([128, ST], F32, tag="hif")
                nc.vector.tensor_copy(hif[:], hi32[:])
                lof = idxp.tile([128, ST], F32, tag="lof")
                nc.vector.tensor_copy(lof[:], lo32[:])

                if alimit < 6:
                    continue
                ps_sc = psp.tile([HI, LO], F32, tag="pssc")
                for k in range(ST):
                    gp = f16p.tile([128, HI], F16, tag="gp")
                    nc.vector.tensor_scalar(gp[:], iota_hi[:], hif[:, k:k + 1],
                                            None, ALU.is_equal)
                    nc.vector.tensor_scalar(gp[:], gp[:], a_cols[:, k:k + 1],
                                            None, ALU.mult)
                    lp = f16p.tile([128, LO], F16, tag="lp")
                    nc.gpsimd.tensor_scalar(lp[:], iota_lo[:], lof[:, k:k + 1],
                                            None, ALU.is_equal)
                    nc.tensor.matmul(ps_sc[:], lhsT=gp[:], rhs=lp[:],
                                     start=(k == 0), stop=(k == ST - 1))
                nc.vector.tensor_copy(scat_sb[:HI, LO * b:LO * (b + 1)], ps_sc[:])

            # ======== LSTM (all 8 local batches at once) ========
            if phases >= 2:
                acts_sb = constp.tile([128, 16 * BL], F32)
                for g in range(16):
                    ps_g = ps2p.tile([128, BL], F32, tag="small")
                    for f in range(ET + 2):
                        wt = wtp.tile([128, 128], F32, tag="wih")
                        nc.sync.dma_start(
                            wt[:], wihT_d[128 * f:128 * (f + 1),
                                          128 * g:128 * (g + 1)])
                        nc.tensor.matmul(ps_g[:], lhsT=wt[:],
                                         rhs=xT_sb[:, BL * f:BL * (f + 1)],
                                         start=(f == 0), stop=False)
                    for f in range(HT):
                        wt2 = wtp.tile([128, 128], F32, tag="whh")
                        nc.sync.dma_start(
                            wt2[:], whhT_d[128 * f:128 * (f + 1),
                                           128 * g:128 * (g + 1)])
                        nc.tensor.matmul(ps_g[:], lhsT=wt2[:],
                                         rhs=h0T_sb[:, BL * f:BL * (f + 1)],
                                         start=False, stop=(f == HT - 1))
                    func = AF.Tanh if 8 <= g < 12 else AF.Sigmoid
                    nc.scalar.activation(acts_sb[:, BL * g:BL * (g + 1)], ps_g[:],
                                         func, bias=lstmb_sb[:, g:g + 1])

                htT_sb = constp.tile([128, HT * BL], F32)
                ctT_sb = constp.tile([128, HT * BL], F32)
                for j in range(HT):
                    i_g = acts_sb[:, BL * j:BL * (j + 1)]
                    f_g = acts_sb[:, BL * (4 + j):BL * (5 + j)]
                    g_g = acts_sb[:, BL * (8 + j):BL * (9 + j)]
                    o_g = acts_sb[:, BL * (12 + j):BL * (13 + j)]
                    ct = ctT_sb[:, BL * j:BL * (j + 1)]
                    nc.vector.tensor_tensor(out=ct, in0=f_g,
                                            in1=c0T_sb[:, BL * j:BL * (j + 1)],
                                            op=ALU.mult)
                    ig = rowp.tile([128, BL], F32, tag="ig")
                    nc.vector.tensor_tensor(out=ig[:], in0=i_g, in1=g_g,
                                            op=ALU.mult)
                    nc.vector.tensor_tensor(out=ct, in0=ct, in1=ig[:], op=ALU.add)
                    tct = rowp.tile([128, BL], F32, tag="tct")
                    nc.scalar.activation(tct[:], ct, AF.Tanh)
                    nc.vector.tensor_tensor(out=htT_sb[:, BL * j:BL * (j + 1)],
                                            in0=o_g, in1=tct[:], op=ALU.mult)

                # p_gen = sigmoid(ctx.whv + h_t.wsv + emb.wxv)
                ps_pg = ps2p.tile([1, BL], F32, tag="small")
                for e in range(ET):
                    nc.tensor.matmul(ps_pg[:], lhsT=whv_sb[:, e:e + 1],
                                     rhs=xT_sb[:, BL * e:BL * (e + 1)],
                                     start=(e == 0), stop=False)
                for j in range(HT):
                    nc.tensor.matmul(ps_pg[:], lhsT=wsv_sb[:, j:j + 1],
                                     rhs=htT_sb[:, BL * j:BL * (j + 1)],
                                     start=False, stop=False)
                for j in range(2):
                    nc.tensor.matmul(ps_pg[:], lhsT=wxv_sb[:, j:j + 1],
                                     rhs=xT_sb[:, BL * (ET + j):BL * (ET + j + 1)],
                                     start=False, stop=(j == 1))
                pg_row = constp.tile([1, BL], F32)
                nc.scalar.activation(pg_row[:], ps_pg[:], AF.Sigmoid)
                nc.scalar.activation(lnpg_row[:], pg_row[:], AF.Ln)
                nc.vector.tensor_scalar(s2_row[:], pg_row[:], -1.0, 1.0,
                                        ALU.mult, ALU.add)

                # h_t / c_t back to natural layout; h_t -> AllGather
                htnat_sb = constp.tile([BL, H], F32)
                for j in range(HT):
                    pst2 = ps2p.tile([BL, 128], F32, tag="small")
                    nc.tensor.transpose(pst2[:], htT_sb[:, BL * j:BL * (j + 1)],
                                        ident[:, :])
                    nc.scalar.copy(htnat_sb[:, 128 * j:128 * (j + 1)], pst2[:])
                nc.sync.dma_start(ht_in[:, :], htnat_sb[:])
                nc.sync.dma_start(outht_d[:, :], htnat_sb[:])
                ctnat_sb = constp.tile([BL, H], F32)
                for j in range(HT):
                    pst4 = ps2p.tile([BL, 128], F32, tag="small")
                    nc.tensor.transpose(pst4[:], ctT_sb[:, BL * j:BL * (j + 1)],
                                        ident[:, :])
                    nc.scalar.copy(ctnat_sb[:, 128 * j:128 * (j + 1)], pst4[:])
                nc.sync.dma_start(outct_d[:, :], ctnat_sb[:])

            if phases >= 3:
                nc.gpsimd.collective_compute(
                    "AllGather", mybir.AluOpType.bypass, replica_groups=groups,
                    ins=[ht_in[:, :].opt()], outs=[ht_out[:, :].opt()])

                # h_t_all^T (bf16) for the logits matmul
                htall_sb = constp.tile([B, H], F32)
                nc.sync.dma_start(htall_sb[:], ht_out[:, :])
                htTall_sb = constp.tile([128, HT * B], BF16)
                for j in range(HT):
                    pst3 = ps2p.tile([128, B], F32, tag="small")
                    nc.tensor.transpose(pst3[:],
                                        htall_sb[:, 128 * j:128 * (j + 1)],
                                        ident[:B, :B])
                    nc.scalar.copy(htTall_sb[:, B * j:B * (j + 1)], pst3[:])

                # logits chunk [64, VC] + exp partial denominators (streamed)
                dpart_sb = constp.tile([B, NVC], F32)
                for vc in range(NVC):
                    ps_l = psp.tile([B, 512], F32, tag="pse")
                    for k in range(HT):
                        vt = wtp.tile([128, 512], BF16, tag="vw")
                        nc.sync.dma_start(vt[:], vwT_d[128 * k:128 * (k + 1),
                                                       512 * vc:512 * (vc + 1)])
                        nc.tensor.matmul(ps_l[:],
                                         lhsT=htTall_sb[:, B * k:B * (k + 1)],
                                         rhs=vt[:], start=(k == 0), stop=False)
                    vbt = wtp.tile([1, 512], BF16, tag="vbs")
                    nc.sync.dma_start(vbt[:], vb_d[:, 512 * vc:512 * (vc + 1)])
                    nc.tensor.matmul(ps_l[:], lhsT=ones_bf[:1, :B],
                                     rhs=vbt[:1, :], start=False, stop=True)
                    lchunk = scrp.tile([B, 512], F32, tag="lchunk")
                    nc.scalar.activation(lchunk[:], ps_l[:], AF.Exp,
                                         accum_out=dpart_sb[:, vc:vc + 1])
                    nc.scalar.copy(lchunk[:], ps_l[:])
                    nc.sync.dma_start(a2a_in[:, 512 * vc:512 * (vc + 1)],
                                      lchunk[:])
                dsum = constp.tile([B, 1], F32)
                nc.vector.tensor_reduce(dsum[:], dpart_sb[:], axis=AX.X,
                                        op=ALU.add)
                nc.sync.dma_start(a2a_in[:, VC:VC + 1], dsum[:])

            if phases >= 4:
                nc.gpsimd.collective_compute(
                    "AllToAll", mybir.AluOpType.bypass, replica_groups=groups,
                    ins=[a2a_in[:, :].opt()], outs=[a2a_out[:, :].opt()])

                # ======== mix + output, per local batch ========
                for lb in range(BL):
                    lg = mixp.tile([HI, LO], F32, tag="lg")
                    for i in range(NCORES):
                        nc.sync.dma_start(
                            lg[13 * i:13 * (i + 1), :],
                            a2a_out[NCORES * i + lb, 0:VC].rearrange(
                                "(r c) -> r c", c=512))
                    den8 = mixp.tile([1, NCORES], F32, tag="den8")
                    nc.sync.dma_start(
                        den8[:1, :],
                        a2a_out[lb::NCORES, VC:VC + 1].rearrange("r o -> o r"))
                    dtot = mixp.tile([1, 1], F32, tag="dtot")
                    nc.vector.tensor_reduce(dtot[:], den8[:], axis=AX.X,
                                            op=ALU.add)
                    lnden = mixp.tile([1, 1], F32, tag="lnden")
                    nc.scalar.activation(lnden[:], dtot[:], AF.Ln)
                    bias_b = mixp.tile([1, 1], F32, tag="biasb")
                    nc.vector.tensor_tensor(out=bias_b[:],
                                            in0=lnpg_row[:, lb:lb + 1],
                                            in1=lnden[:], op=ALU.subtract)
                    bcol = mixp.tile([128, 1], F32, tag="bcol")
                    nc.gpsimd.partition_broadcast(bcol[:], bias_b[:])
                    s2col = mixp.tile([128, 1], F32, tag="s2col")
                    nc.gpsimd.partition_broadcast(s2col[:],
                                                  s2_row[:1, lb:lb + 1])
                    pv = mixp.tile([HI, LO], F32, tag="pv")
                    nc.scalar.activation(pv[:], lg[:], AF.Exp,
                                         bias=bcol[:HI, :1])
                    osb = mixp.tile([HI, LO], F32, tag="osb")
                    nc.vector.scalar_tensor_tensor(
                        out=osb[:], in0=scat_sb[:HI, LO * lb:LO * (lb + 1)],
                        scalar=s2col[:HI, :1], in1=pv[:],
                        op0=ALU.mult, op1=ALU.add)
                    nc.sync.dma_start(
                        out_d[lb, :].rearrange("(r c) -> r c", c=512), osb[:])

    nc.compile()
    return nc


def _host_prep(inputs):
    f32 = np.float32
    enc = np.asarray(inputs["enc_out"], f32)
    h0 = np.asarray(inputs["h0"], f32)
    c0 = np.asarray(inputs["c0"], f32)
    dec = np.asarray(inputs["dec_input"]).astype(np.int32)
    eidx = np.asarray(inputs["enc_inputs"]).astype(np.int32)
    emb = np.ascontiguousarray(np.asarray(inputs["embed_table"], f32))
    whwT = np.ascontiguousarray(np.asarray(inputs["attn_wh_w"], f32).T)
    wswT = np.ascontiguousarray(np.asarray(inputs["attn_ws_w"], f32).T)
    attnb = np.ascontiguousarray(
        (np.asarray(inputs["attn_wh_b"], f32)
         + np.asarray(inputs["attn_ws_b"], f32)).reshape(H, 1))
    attnv = np.ascontiguousarray(np.asarray(inputs["attn_v"], f32).reshape(H, 1))
    wihT = np.ascontiguousarray(np.asarray(inputs["lstm_w_ih"], f32).T)
    whhT = np.ascontiguousarray(np.asarray(inputs["lstm_w_hh"], f32).T)
    lstmb = np.ascontiguousarray(
        (np.asarray(inputs["lstm_b_ih"], f32)
         + np.asarray(inputs["lstm_b_hh"], f32)).reshape(4 * H, 1))
    whv = np.ascontiguousarray(np.asarray(inputs["wh_vec"], f32).reshape(E, 1))
    wsv = np.ascontiguousarray(np.asarray(inputs["ws_vec"], f32).reshape(H, 1))
    wxv = np.ascontiguousarray(np.asarray(inputs["wx_vec"], f32).reshape(EMB, 1))
    vw = np.asarray(inputs["v_w"], f32)            # [V, H]
    vb = np.asarray(inputs["v_b"], f32)            # [V]

    vwT_pad = np.zeros((H, VP), f32)
    vwT_pad[:, :V] = vw.T
    vb_pad = np.full((VP,), NEG, f32)
    vb_pad[:V] = vb
    vwT_bf = vwT_pad.astype(ml_dtypes.bfloat16)
    vb_bf = vb_pad.astype(ml_dtypes.bfloat16).reshape(1, VP)

    in_maps = []
    for c in range(NCORES):
        bs = slice(BL * c, BL * (c + 1))
        vs = slice(VC * c, VC * (c + 1))
        in_maps.append({
            "encT": np.ascontiguousarray(enc[bs].transpose(0, 2, 1)),
            "h0T": np.ascontiguousarray(h0[bs].T),
            "c0T": np.ascontiguousarray(c0[bs].T),
            "decin": np.ascontiguousarray(dec[bs].reshape(BL, 1)),
            "encidx": np.ascontiguousarray(eidx[bs]),
            "embed": emb,
            "whwT": whwT, "wswT": wswT, "attnb": attnb, "attnv": attnv,
            "wihT": wihT, "whhT": whhT, "lstmb": lstmb,
            "whv": whv, "wsv": wsv, "wxv": wxv,
            "vwT": np.ascontiguousarray(vwT_bf[:, vs]),
            "vb": np.ascontiguousarray(vb_bf[:, vs]),
        })
    return in_maps


def kernel(**inputs):
    from concourse.bass_utils import run_bass_kernel_spmd

    if "nc" not in _CACHE:
        _CACHE["nc"] = _build_module()
    nc = _CACHE["nc"]
    in_maps = _host_prep(inputs)
    res = run_bass_kernel_spmd(nc, in_maps, core_ids=list(range(NCORES)))
    out = np.concatenate([res.results[c]["out"] for c in range(NCORES)], axis=0)
    h_t = np.concatenate([res.results[c]["out_ht"] for c in range(NCORES)], axis=0)
    c_t = np.concatenate([res.results[c]["out_ct"] for c in range(NCORES)], axis=0)
    # mirror the reference's return structure: (output, h_t, c_t)
    return (np.ascontiguousarray(out[:, :V]).astype(np.float32),
            h_t.astype(np.float32), c_t.astype(np.float32))
